# revision 34
# baseline (speedup 1.0000x reference)
"""Trainium2 Bass kernel for the NeuralMemory (scatter_memory) problem.

Math (B=1, N=512, D=128, DEPTH=4): per-token meta-gradients of the memory
MLP are rank-1 per layer, so the (n, depth, d, d) momentum/update scans
collapse to a scalar coefficient matrix C[t,s] applied attention-style:

    retrieved_l(t) = y_t @ W_l + sum_s C[t,s]*(-lr_s) * (y_t . x_l(s)) * g_l(s)

C is numerically banded (C[t,s]==0 in fp32 for t-s>=64), so each of the 8
cores handles one 64-query window with a 128-token key window -- fully
data-parallel, no collectives.  Core 0's missing past is zero-padded.

v7 latency notes (the kernel is latency-bound; every engine <55% busy):
  - Input DMA completion semaphores land ~1.9us after the DMA instruction
    retires (HBM read receipt round-trip).  The input is split into 4
    chunks on the SP HWDGE ring ordered by first use, the first chunk
    minimal (seqW + host-fused wk0) so the chain starts ~6.1us.
  - h1 = (Wk@W0)^T @ seq^T via host-fused wk0; x0 (keys^T) is computed
    off-chain later, only for S0.
  - d4 = h4 - v^T forms inside one PSUM accumulation group (host-negated
    Wv start matmul runs early), then one Scalar copy to SBUF.
  - silu' chains: sp1/sp2 on GpSimd (4-op form), sp3 on DVE (3-op STT,
    back-to-back same-engine) so it lands just after b3 = wmT3 @ d4.
  - bb = 0.5-0.5*th_dec via a Scalar Copy activation (scale/bias) so the
    CT scan never head-of-line blocks the DVE delta chain.
  - The Tile scheduler's cost model is optimistic about DMA completion,
    so DMA-gated projection matmuls (lrb/amb/dec/x0/q) are allocated from
    the SAME rotating 2-slot PSUM pool as the chain's h tiles: slot WAR
    dependencies pin the static order in both the scheduler's model and
    hardware (they fill real PE gaps instead of stalling the chain).
  - Dummy warm matmuls keep the PE busy from ~4.1us toward the HAM clock
    gate's 3.4us activity window (1.2 -> 2.4 GHz).
"""

import numpy as np

D = 128
N = 512
DEPTH = 4
NCORES = 8
QW = N // NCORES        # 64 queries per core
SW = 2 * QW             # 128-token key window per core

# column offsets inside the consolidated per-core input tensor (128, ALLW).
OFF_SEQW = 0                     # chunk 1 (SP ring) [0:256)
OFF_WK0 = 128                    # (Wk @ W_mem[0]) host-fused
OFF_WVN = 256                    # chunk 2 (SP ring) [256:768): -Wv
OFF_WM1 = 384
OFF_WM2 = 512
OFF_WM3 = 640
OFF_REPL = 768                   # chunk 3 (SP ring) [768:1536): reps first
OFF_REPM = 896
OFF_REPD = 1024
OFF_ID = 1152
OFF_WQ = 1280                    # wq/wk last: late sim-subtile readiness
OFF_WK = 1408
OFF_WMT = 1536                   # chunk 4 (ACT ring) [1536:2176): W^T x4
OFF_WM0 = 2048
ALLW = 2176

_cache = {}


def _build_program():
    import concourse.mybir as mybir
    from concourse import bacc
    from concourse.tile import TileContext

    f32 = mybir.dt.float32
    fp16 = mybir.dt.float16
    AF = mybir.ActivationFunctionType
    ALU = mybir.AluOpType

    nc = bacc.Bacc("TRN2")

    allin_d = nc.dram_tensor("allin", [D, ALLW], fp16, kind="ExternalInput")
    outT_d = nc.dram_tensor("outT", [D, QW], fp16, kind="ExternalOutput")

    with TileContext(nc) as tc:
        with (
            tc.tile_pool(name="sb", bufs=1) as sb,
            tc.tile_pool(name="ph", bufs=2, space="PSUM") as ph,  # chain+proj
            tc.tile_pool(name="pm", bufs=2, space="PSUM") as pm,  # d4,S*
            tc.tile_pool(name="pb", bufs=2, space="PSUM") as pb,  # b*,tr*
            tc.tile_pool(name="pr", bufs=1, space="PSUM") as pr,  # warm,r*
        ):
            def sbt(tag, shape=(D, SW), dt=fp16):
                return sb.tile(list(shape), dt, tag=tag, name=tag)

            allin = sbt("allin", (D, ALLW))
            # SP ring: 3 chunks ordered by first use; ACT ring: wmT chunk.
            nc.sync.dma_start(out=allin[:, 0:256], in_=allin_d[:, 0:256])
            nc.sync.dma_start(out=allin[:, 256:768], in_=allin_d[:, 256:768])
            nc.sync.dma_start(out=allin[:, 1536:2176],
                              in_=allin_d[:, 1536:2176])
            nc.sync.dma_start(out=allin[:, 768:1536], in_=allin_d[:, 768:1536])

            seqW = allin[:, OFF_SEQW:OFF_SEQW + SW]
            wk0 = allin[:, OFF_WK0:OFF_WK0 + D]
            wvn = allin[:, OFF_WVN:OFF_WVN + D]
            wm = [allin[:, OFF_WM0:OFF_WM0 + D],
                  allin[:, OFF_WM1:OFF_WM1 + D],
                  allin[:, OFF_WM2:OFF_WM2 + D],
                  allin[:, OFF_WM3:OFF_WM3 + D]]
            wq = allin[:, OFF_WQ:OFF_WQ + D]
            wk = allin[:, OFF_WK:OFF_WK + D]
            rep_lr = allin[:, OFF_REPL:OFF_REPL + D]
            rep_mom = allin[:, OFF_REPM:OFF_REPM + D]
            rep_dec = allin[:, OFF_REPD:OFF_REPD + D]
            idm = allin[:, OFF_ID:OFF_ID + D]
            wmT = [allin[:, OFF_WMT + D * l:OFF_WMT + D * (l + 1)]
                   for l in range(DEPTH)]

            # ---- PE clock warmup until chunk 1's completion (~6.1us) ----
            scrw = sbt("scrw", (D, D))
            nc.vector.memset(scrw, 0.0)
            warm = pr.tile([D, D], f32, tag="r", name="warm")
            # HAM never un-throttles in practice (forward-chain gaps reset
            # its window); 3 warms suffice to fill the DMA wait + feed the
            # wsink verifier read, and 16 fewer instructions shrink the
            # engine TENSOR_LOAD preamble.
            for _ in range(3):
                nc.tensor.matmul(warm, scrw, scrw, start=True, stop=True)
            wsink = sbt("wsink", (D, 8))
            nc.vector.tensor_copy(wsink, warm[:, 0:8])

            # ---- forward chain.  Every DMA-gated projection matmul is
            # allocated from the SAME rotating PSUM pool as the chain's h
            # tiles, so slot WAR dependencies pin the static order (the
            # scheduler's optimistic DMA model can no longer hoist them
            # into the chain's path).
            ps_h1 = ph.tile([D, SW], f32, tag="h", name="h1")
            nc.tensor.matmul(ps_h1, wk0, seqW, start=True, stop=True)
            # d4 group: -v^T start matmul early, h4 accumulates later
            ps_d4 = pm.tile([D, SW], f32, tag="m", name="d4")
            nc.tensor.matmul(ps_d4, wvn, seqW, start=True, stop=False)
            x1 = sbt("x1")
            nc.scalar.activation(x1, ps_h1, AF.Silu)

            ps_h2 = ph.tile([D, SW], f32, tag="h", name="h2")
            nc.tensor.matmul(ps_h2, wm[1], x1, start=True, stop=True)
            x2 = sbt("x2")
            nc.scalar.activation(x2, ps_h2, AF.Silu)
            th2 = sbt("th2")
            nc.scalar.activation(th2, ps_h2, AF.Tanh, scale=0.5)
            # th1 after th2: sp1 has 2.7us of slack, x2 does not
            th1 = sbt("th1")
            nc.scalar.activation(th1, ps_h1, AF.Tanh, scale=0.5)

            ps_h3 = ph.tile([D, SW], f32, tag="h", name="h3")
            nc.tensor.matmul(ps_h3, wm[2], x2, start=True, stop=True)
            x3 = sbt("x3")
            nc.scalar.activation(x3, ps_h3, AF.Silu)
            th3 = sbt("th3")
            nc.scalar.activation(th3, ps_h3, AF.Tanh, scale=0.5)

            # h4 accumulates onto -v^T: ps_d4 becomes d4 = h4 - v^T
            nc.tensor.matmul(ps_d4, wm[3], x3, start=False, stop=True)
            d4 = sbt("d4")
            nc.scalar.copy(d4, ps_d4)

            # sp1/sp2 on GpSimd (4-op form): s = 0.5+0.5*th; sp = s+x-x*s
            def sp_gp(th, x, pref):
                s = sbt(f"{pref}s")
                nc.gpsimd.tensor_scalar(s, th, 0.5, 0.5, ALU.mult, ALU.add)
                xs = sbt(f"{pref}xs")
                nc.gpsimd.tensor_mul(xs, x, s)
                u = sbt(f"{pref}u")
                nc.gpsimd.tensor_sub(u, x, xs)
                sp = sbt(f"{pref}sp")
                nc.gpsimd.tensor_add(sp, s, u)
                return sp

            sp2 = sp_gp(th2, x2, "s2")
            sp1 = sp_gp(th1, x1, "s1")

            # projections: slot-chained behind h2/h3 readers; amb goes
            # behind izlr (lrb's reader) so the AT scan can't jump the
            # sp3 chain in the static DVE order
            ps_lrb = ph.tile([D, SW], f32, tag="h", name="lrb")
            nc.tensor.matmul(ps_lrb, rep_lr, seqW, start=True, stop=True)
            ps_dec = ph.tile([D, SW], f32, tag="h", name="dec")
            nc.tensor.matmul(ps_dec, rep_dec, seqW, start=True, stop=True)

            # DVE: izlr, then sp3 back-to-back (same-engine, no sem gaps)
            izlr = sbt("izlr")
            nc.vector.tensor_mul(izlr, idm, ps_lrb)
            s3t = sbt("s3t")
            nc.vector.scalar_tensor_tensor(s3t, th3, -0.5, x3,
                                           ALU.mult, ALU.mult)
            s3w = sbt("s3w")
            nc.vector.scalar_tensor_tensor(s3w, th3, 1.0, x3,
                                           ALU.add, ALU.add)
            sp3 = sbt("sp3")
            nc.vector.scalar_tensor_tensor(sp3, s3w, 0.5, s3t,
                                           ALU.mult, ALU.add)

            ps_amb = ph.tile([D, SW], f32, tag="h", name="amb")
            nc.tensor.matmul(ps_amb, rep_mom, seqW, start=True, stop=True)
            AT = sbt("AT")
            nc.vector.tensor_tensor_scan(AT, ps_amb, izlr, 0.0,
                                         ALU.mult, ALU.add)
            th_dec = sbt("th_dec")
            nc.scalar.activation(th_dec, ps_dec, AF.Tanh, scale=0.5)
            # bb = 0.5 - 0.5*th_dec on Scalar (Copy with scale+bias); the
            # GpSimd queue is busy with the sp2 chain and would starve the
            # CT scan (which head-of-line blocks the DVE delta chain)
            bb = sbt("bb")
            nc.scalar.activation(bb, th_dec, AF.Copy, bias=0.5, scale=-0.5)

            # ---- backward delta chain ----
            ps_b3 = pb.tile([D, SW], f32, tag="bt", name="b3")
            nc.tensor.matmul(ps_b3, wmT[3], d4, start=True, stop=True)
            d3 = sbt("d3")
            nc.vector.tensor_mul(d3, ps_b3, sp3)
            ps_b2 = pb.tile([D, SW], f32, tag="bt", name="b2")
            nc.tensor.matmul(ps_b2, wmT[2], d3, start=True, stop=True)
            d2 = sbt("d2")
            nc.vector.tensor_mul(d2, ps_b2, sp2)
            ps_b1 = pb.tile([D, SW], f32, tag="bt", name="b1")
            nc.tensor.matmul(ps_b1, wmT[1], d2, start=True, stop=True)
            d1 = sbt("d1")
            nc.vector.tensor_mul(d1, ps_b1, sp1)
            CT = sbt("CT")
            nc.vector.tensor_tensor_scan(CT, bb, AT, 0.0, ALU.mult, ALU.add)

            # x0/q: slot-chained behind amb/dec readers
            ps_x0 = ph.tile([D, SW], f32, tag="h", name="x0")
            nc.tensor.matmul(ps_x0, wk, seqW, start=True, stop=True)
            ps_q = ph.tile([D, QW], f32, tag="h", name="q")
            nc.tensor.matmul(ps_q, wq, seqW[:, QW:SW], start=True,
                             stop=True)

            # x0/qT staging on Scalar (DVE is saturated mid-kernel)
            x0 = sbt("x0")
            nc.scalar.copy(x0, ps_x0)
            qT = sbt("qT", (D, QW))
            nc.scalar.copy(qT, ps_q)

            Dl = {1: d1, 2: d2, 3: d3, 4: d4}
            X = [x0, x1, x2, x3]

            # ---- retrieval; G_l transposed lazily, most-critical first ----
            Y = qT
            CTq = CT[:, QW:SW]
            for l in range(DEPTH):
                ps_t = pb.tile([D, D], fp16, tag="bt", name=f"t{l}")
                nc.tensor.transpose(ps_t, Dl[l + 1], idm)
                gl = sbt(f"g{l}")
                nc.vector.tensor_copy(gl, ps_t)
                ps_s = pm.tile([D, QW], f32, tag="m", name=f"S{l}")
                nc.tensor.matmul(ps_s, X[l], Y, start=True, stop=True)
                cst = sbt(f"cst{l}", (D, QW))
                nc.vector.tensor_mul(cst, ps_s, CTq)
                ps_o = pr.tile([D, QW], f32, tag="r", name=f"r{l}")
                nc.tensor.matmul(ps_o, wm[l], Y, start=True, stop=False)
                nc.tensor.matmul(ps_o, gl, cst, start=False, stop=True)
                if l < DEPTH - 1:
                    ynext = sbt(f"y{l + 1}", (D, QW))
                    nc.scalar.activation(ynext, ps_o, AF.Silu)
                    Y = ynext
                else:
                    # single copy + single DMA: the completion latency is
                    # fixed, so the LAST issue time is what gates exec end
                    outT = sbt("outT", (D, QW), dt=fp16)
                    nc.vector.tensor_copy(outT, ps_o)
                    nc.sync.dma_start(out=outT_d[:, 0:QW], in_=outT)

    return nc


def get_program():
    if "nc" not in _cache:
        nc = _build_program()
        nc.finalize()
        _cache["nc"] = nc
    return _cache["nc"]


def make_in_maps(seq, W_mem, W_q, W_kv, W_mom, W_step, W_decay):
    seq = np.asarray(seq, dtype=np.float32)
    W_mem = np.asarray(W_mem, dtype=np.float32)
    W_kv = np.asarray(W_kv, dtype=np.float32)
    seqT = seq.reshape(N, D).T  # (d, n)

    base = np.zeros((D, ALLW), dtype=np.float16)
    base[:, OFF_WK0:OFF_WK0 + D] = W_kv[:, :D] @ W_mem[0]
    base[:, OFF_WVN:OFF_WVN + D] = -W_kv[:, D:]
    base[:, OFF_WK:OFF_WK + D] = W_kv[:, :D]
    base[:, OFF_WQ:OFF_WQ + D] = np.asarray(W_q, dtype=np.float32)
    for l in range(DEPTH):
        off = [OFF_WM0, OFF_WM1, OFF_WM2, OFF_WM3][l]
        base[:, off:off + D] = W_mem[l]
        base[:, OFF_WMT + D * l:OFF_WMT + D * (l + 1)] = W_mem[l].T
    lr_col = np.asarray(W_step, dtype=np.float32)[:, 0] * (-2.0 / D)
    base[:, OFF_REPL:OFF_REPL + D] = np.repeat(lr_col[:, None], D, axis=1)
    base[:, OFF_REPM:OFF_REPM + D] = np.repeat(
        np.asarray(W_mom, dtype=np.float32)[:, :1], D, axis=1)
    base[:, OFF_REPD:OFF_REPD + D] = np.repeat(
        np.asarray(W_decay, dtype=np.float32)[:, :1], D, axis=1)
    base[:, OFF_ID:OFF_ID + D] = np.eye(D, dtype=np.float32)

    in_maps = []
    for c in range(NCORES):
        allin = base.copy()
        qc = c * QW
        lo = qc - QW
        win = np.zeros((D, SW), dtype=np.float16)
        src_lo = max(lo, 0)
        win[:, src_lo - lo:] = seqT[:, src_lo:qc + QW].astype(np.float16)
        allin[:, OFF_SEQW:OFF_SEQW + SW] = win
        in_maps.append({"allin": allin})
    return in_maps


def assemble(results):
    out = np.empty((N, D), dtype=np.float32)
    for c in range(NCORES):
        out[c * QW:(c + 1) * QW, :] = results[c]["outT"].T.astype(np.float32)
    return out.reshape(1, N, D)


def kernel(**inputs) -> np.ndarray:
    from concourse.bass_utils import run_bass_kernel_spmd

    nc = get_program()
    in_maps = make_in_maps(**inputs)
    res = run_bass_kernel_spmd(nc, in_maps, list(range(NCORES)))
    return assemble(res.results)
